# revision 4
# baseline (speedup 1.0000x reference)
"""Trainium2 Bass kernel for an 8-expert top-2 MoE layer (nn_MoE_8383776161864).

Strategy: sparse expert-parallel dispatch. The reference's dense gate-masked
MoE is math-identical to top-2 sparse dispatch, which needs only 1/4 of the
dense FLOPs. Routing (logits -> top-2 -> softmax gates) runs on the host in
exact fp32 (67 MFLOP, negligible); each of the 8 NeuronCores owns one
expert and computes, for the <=C tokens routed to it,

    y_e = gelu(x_e @ w_fc[e].T) @ w_proj[e].T        (two bf16 GEMMs)

The gate weighting and the per-token combine of the two expert contributions
happen on the host (pure gathers + axpy). Device work per core: 2 x 2 x
512*1024*C FLOP ~= 2.4 GFLOP in bf16 (f32 PSUM accumulate), ~6.7 MB of DMA.

Capacity C = 1152 per expert (actual per-expert loads for the fixed seed are
974..1071, mean 1024). If an expert ever receives more than C tokens, the
overflow assignments are computed exactly on the host, so the kernel stays
correct for arbitrary inputs.

Numerics (validated in numpy against an fp64 reference): bf16 inputs/weights
with fp32 PSUM give rel err ~3.6e-3 end to end (tolerance 2e-2). fp8 (even
with per-tensor scaling) measures 3.4e-2+ and is not viable.

All shapes are hardcoded; kernel() takes the full (unsharded) inputs:
    hidden_states [2, 2048, 1024] f32
    w_gate  [8, 1024] f32
    w_fc    [8, 512, 1024] f32
    w_proj  [8, 1024, 512] f32
and returns the full [2, 2048, 1024] f32 output.
"""

import os
import sys

import numpy as np
import ml_dtypes

BF16 = ml_dtypes.bfloat16

E = 8
H = 1024
I = 512
B, S = 2, 2048
T = B * S
TOP_K = 2
NCORES = 8
KT = H // 128  # 8 k-tiles over H
IT = I // 128  # 4 k-tiles over I

C = 1152  # per-expert token capacity on device
CHUNKS = [(0, 512), (512, 512), (1024, 128)]  # (start, size) over C

_cache = {}


def _import_concourse():
    try:
        import concourse  # noqa: F401
    except ImportError:
        for p in ("/opt/trn_rl_repo", "/root/.axon_site/_ro/trn_rl_repo"):
            if os.path.isdir(p) and p not in sys.path:
                sys.path.insert(0, p)
        import concourse  # noqa: F401


def build_nc():
    """Per-core Bass module: y[:, 0:C] = gelu(x @ wfc.T) @ wpr.T in bf16.

    DRAM layout (all bf16):
      xk  [128, KT*C]    x^T k-tiles, chunk-major: block (j, k) holds
                         x^T[k*128 + p, c0_j : c0_j + F_j]
      wfc [128, KT, I]   wfc[p, k, i]  = w_fc[i, k*128 + p]
      wpr [128, IT, H]   wpr[p, kk, h] = w_proj[h, kk*128 + p]
      yk  [128, KT, C]   yk[p, m, c]   = y[c, m*128 + p]
    """
    _import_concourse()
    import concourse.tile as tile
    from concourse import bacc, mybir

    f32 = mybir.dt.float32
    bf16 = mybir.dt.bfloat16

    nc = bacc.Bacc(None, target_bir_lowering=False, debug=False)

    xk = nc.dram_tensor("xk", [128, KT * C], bf16, kind="ExternalInput")
    wfc = nc.dram_tensor("wfc", [128, KT, I], bf16, kind="ExternalInput")
    wpr = nc.dram_tensor("wpr", [128, IT, H], bf16, kind="ExternalInput")
    yk = nc.dram_tensor("yk", [128, KT, C], bf16, kind="ExternalOutput")

    with tile.TileContext(nc) as tc:
        with (
            tc.tile_pool(name="xp", bufs=1) as xp,
            tc.tile_pool(name="wp", bufs=1) as wp,
            tc.tile_pool(name="hmp", bufs=2) as hmp,
            tc.tile_pool(name="yp", bufs=4) as yp,
            tc.tile_pool(name="pp", bufs=1, space="PSUM") as pp,
        ):
            x_sb = xp.tile([128, KT, C], bf16)
            wfc_sb = wp.tile([128, KT, I], bf16)
            wpr_sb = wp.tile([128, IT, H], bf16)

            # DMA issue order matches consumption order so the PE can start
            # after the first (x k-tile, wfc k-tile) pair lands.
            off = 0
            for j, (c0, F) in enumerate(CHUNKS):
                for k in range(KT):
                    nc.sync.dma_start(x_sb[:, k, c0 : c0 + F], xk[:, off : off + F])
                    off += F
                    if j == 0:
                        nc.sync.dma_start(wfc_sb[:, k, :], wfc[:, k, :])
                if j == 0:
                    nc.sync.dma_start(wpr_sb[:, :, 0:512], wpr[:, :, 0:512])
                if j == 1:
                    nc.sync.dma_start(wpr_sb[:, :, 512:1024], wpr[:, :, 512:1024])

            for j, (c0, F) in enumerate(CHUNKS):
                # mm1, k-outer: 4 PSUM accumulation groups (one per I m-tile)
                # stay open across the k loop so compute starts on k-tile 0.
                pms = []
                for m in range(IT):
                    pm1 = pp.tile(
                        [128, 512], f32, tag=f"p1_{m}", bufs=1, name=f"pm1_{j}_{m}"
                    )
                    pms.append(pm1)
                for k in range(KT):
                    for m in range(IT):
                        nc.tensor.matmul(
                            pms[m][:, :F],
                            wfc_sb[:, k, m * 128 : (m + 1) * 128],
                            x_sb[:, k, c0 : c0 + F],
                            start=(k == 0),
                            stop=(k == KT - 1),
                        )
                hm = hmp.tile([128, IT, 512], bf16, tag="hm", name=f"hm_{j}")
                for m in range(IT):
                    nc.scalar.activation(
                        hm[:, m, :F], pms[m][:, :F], mybir.ActivationFunctionType.Gelu
                    )
                # mm2, m-outer: sequential groups through a 4-deep PSUM ring.
                for m in range(KT):
                    pm2 = pp.tile(
                        [128, 512], f32, tag="p2", bufs=4, name=f"pm2_{j}_{m}"
                    )
                    for kk in range(IT):
                        nc.tensor.matmul(
                            pm2[:, :F],
                            wpr_sb[:, kk, m * 128 : (m + 1) * 128],
                            hm[:, kk, :F],
                            start=(kk == 0),
                            stop=(kk == IT - 1),
                        )
                    yt = yp.tile([128, 512], bf16, tag="y", name=f"y_{j}_{m}")
                    nc.vector.tensor_copy(yt[:, :F], pm2[:, :F])
                    nc.sync.dma_start(yk[:, m, c0 : c0 + F], yt[:, :F])

    nc.compile()
    return nc


def _gelu_f64(v):
    try:
        from scipy.special import erf

        return 0.5 * v * (1.0 + erf(v / np.sqrt(2.0)))
    except ImportError:
        # tanh approximation fallback (only used for rare overflow tokens)
        return (
            0.5 * v * (1.0 + np.tanh(np.sqrt(2.0 / np.pi) * (v + 0.044715 * v**3)))
        )


def prepare(hidden_states, w_gate, w_fc, w_proj):
    """Host routing + dispatch. Returns (in_maps, meta)."""
    x = np.asarray(hidden_states, dtype=np.float32).reshape(T, H)
    wg = np.asarray(w_gate, dtype=np.float32)
    wfc_f = np.asarray(w_fc, dtype=np.float32)
    wpr_f = np.asarray(w_proj, dtype=np.float32)

    # --- routing (exact fp32, matches the jax reference) ---
    logits = x @ wg.T  # [T, E]
    top2 = np.argsort(-logits, axis=1, kind="stable")[:, :TOP_K]  # [T, 2]
    vals = np.take_along_axis(logits, top2, axis=1)
    ex = np.exp(vals - vals[:, :1])
    gates = ex / ex.sum(axis=1, keepdims=True)  # [T, 2] fp32

    # --- group assignments by expert ---
    flat_e = top2.ravel()  # assignment a = 2*t + kslot -> expert
    order = np.argsort(flat_e, kind="stable")
    counts = np.bincount(flat_e, minlength=E)
    starts = np.concatenate(([0], np.cumsum(counts)))
    pos = np.empty(2 * T, dtype=np.int64)
    pos[order] = np.arange(2 * T) - starts[flat_e[order]]  # slot within expert

    x_bf = x.astype(BF16)
    in_maps = []
    tok_lists = []
    for e in range(E):
        toks = order[starts[e] : starts[e] + min(counts[e], C)] // 2
        tok_lists.append(toks)
        idx = np.concatenate([toks, np.zeros(C - len(toks), dtype=np.int64)])
        gT = np.ascontiguousarray(x_bf[idx].T)  # [H, C] bf16
        blocks = []
        for c0, F in CHUNKS:
            blk = (
                gT[:, c0 : c0 + F]
                .reshape(KT, 128, F)
                .transpose(1, 0, 2)
                .reshape(128, KT * F)
            )
            blocks.append(blk)
        xk_np = np.ascontiguousarray(np.concatenate(blocks, axis=1))
        wfc_np = np.ascontiguousarray(
            wfc_f[e].T.reshape(KT, 128, I).transpose(1, 0, 2)
        ).astype(BF16)
        wpr_np = np.ascontiguousarray(
            wpr_f[e].T.reshape(IT, 128, H).transpose(1, 0, 2)
        ).astype(BF16)
        in_maps.append({"xk": xk_np, "wfc": wfc_np, "wpr": wpr_np})

    meta = {
        "x": x,
        "wfc_f": wfc_f,
        "wpr_f": wpr_f,
        "top2": top2,
        "gates": gates,
        "flat_e": flat_e,
        "pos": pos,
        "counts": counts,
    }
    return in_maps, meta


def combine(yks, meta):
    """yks: list of 8 per-core yk arrays [128, KT, C]. Returns [B, S, H] f32."""
    Y = np.stack([np.asarray(y) for y in yks])  # [E, 128, KT, C]
    YT = (
        Y.transpose(0, 2, 1, 3).reshape(E, H, C).astype(np.float32)
    )  # [E, H, C] (h = m*128 + p)

    flat_e, pos, gates = meta["flat_e"], meta["pos"], meta["gates"]
    x, wfc_f, wpr_f = meta["x"], meta["wfc_f"], meta["wpr_f"]

    slot = np.minimum(pos, C - 1)
    contrib = YT[flat_e, :, slot]  # [2T, H] f32

    # exact host fallback for overflow assignments (pos >= C)
    ov = np.nonzero(pos >= C)[0]
    if len(ov):
        for e in range(E):
            a = ov[flat_e[ov] == e]
            if len(a) == 0:
                continue
            toks = a // 2
            hmo = _gelu_f64(
                x[toks].astype(np.float64) @ wfc_f[e].T.astype(np.float64)
            )
            contrib[a] = (hmo @ wpr_f[e].T.astype(np.float64)).astype(np.float32)

    out = (gates.reshape(-1, 1) * contrib).reshape(T, TOP_K, H).sum(axis=1)
    return out.reshape(B, S, H).astype(np.float32)


def run(in_maps, trace=False):
    _import_concourse()
    from concourse.bass_utils import run_bass_kernel_spmd

    if "nc" not in _cache:
        _cache["nc"] = build_nc()
    nc = _cache["nc"]
    return run_bass_kernel_spmd(
        nc, in_maps, core_ids=list(range(NCORES)), trace=trace
    )


def kernel(hidden_states, w_gate, w_fc, w_proj):
    in_maps, meta = prepare(hidden_states, w_gate, w_fc, w_proj)
    res = run(in_maps, trace=False)
    return combine([res.results[c]["yk"] for c in range(NCORES)], meta)
